# revision 3
# baseline (speedup 1.0000x reference)
"""Causal multi-head self-attention on 8 Trainium2 NeuronCores (v2).

Problem: x[4,2048,1024], Wq/Wk/Wv/Wo[1024,1024], H=16 heads, dk=64.
Sharding: core c handles batch b=c//2 and head-half hh=c%2 (8 heads).
Each core returns a partial output; the host sums core pairs.

v2 dtype plan (cost model: fp8 DoubleRow matmul = 0.5 cyc/row, bf16 =
1.0, fp32r = 1.0 at N>=256):
  - Q/K projections: fp8e4m3 DoubleRow over k-tile pairs (x8, wq8/wk8).
  - V projection: 3-term fp8 DR (x8*wv8 + x8*wv2 + x2*wv8) where *2 are
    host-computed fp8 residuals -> v accurate to ~0.1%.
  - scores: DR with lhsT=(k,k) stride-0 broadcast slots and rhs=(q1,q2)
    split slots -> q effectively ~11-bit, k single fp8.
  - exp on ACT writes e as bf16; PV and O-projection in bf16.
  - normalization: DVE reciprocal + rank-1 PE broadcast, multiply reads
    both PSUM operands directly (no staging copy).
"""

import os
import numpy as np
import ml_dtypes

import concourse.bass as bass
import concourse.mybir as mybir
import concourse.tile as tile
from concourse.bass_utils import run_bass_kernel_spmd
from concourse.vector_clock import ScopedClock, VectorClock

B, S, D, H, DK = 4, 2048, 1024, 16, 64
HPC = H // 2          # heads per core
HD = HPC * DK         # 512 head-dim columns per core
CH = 512              # q-chunk width
NCH = S // CH         # 4
NKB = S // 128        # 16 k-blocks
KC = D // 128         # 8 contraction k-tiles for the projections
F32 = mybir.dt.float32
F32R = mybir.dt.float32r
BF16 = mybir.dt.bfloat16
FP8 = mybir.dt.float8e4
EXP = mybir.ActivationFunctionType.Exp
DR = mybir.MatmulPerfMode.DoubleRow

NP_FP8 = ml_dtypes.float8_e4m3
NP_BF16 = ml_dtypes.bfloat16

# fp8e4m3 normals bottom out at 2^-6; the 0.02-scale weights would land in
# the subnormal range and quantize catastrophically.  Pre-scale weights by
# AL on the host; compensate in the exp scale (q,k both carry AL) and in
# the v ones-column (denominator carries AL like the numerator).  AL=32
# keeps AL*q / AL*k below the 240 max of this fp8e4 flavor when they are
# requantized for the score matmuls.
AL = 32.0

_POOLNORM = os.environ.get("K2_POOLNORM", "0") == "1"
_CAP = float(os.environ.get("K2_CAP", "1e9"))
_PVLAG = int(os.environ.get("K2_PVLAG", "2"))
_PACE = (lambda j: 1.0) if os.environ.get("K2_PACE", "even") == "even" \
    else (lambda j: 1.0 + 0.2 * j)


def _drain_and_barrier_split(self, tick_clock, wait_clock):
    # The stock Tile tail drain attaches every outstanding sem wait to one
    # Drain instruction; this walrus build caps sync waits per instruction
    # and rejects it.  Put each wait on its own SP nop first, then drain
    # with no waits (SP has observed everything by then).
    gc = tick_clock.global_clock
    n = len(gc)
    for proc in range(n):
        t = gc[proc]
        if t == 0:
            continue
        vc = VectorClock([0] * n)
        vc.require_at_least(proc, t)
        nop = self.nc.sync.nop(nofuse=True)
        wait_clock.add_sem_waits(nop.ins, ScopedClock({None: vc}))
    self.nc.sync.drain()
    self.nc.all_engine_barrier()
    assert self.sems is not None
    popped = self.nc._tile_sem_poison_stack.pop()
    assert popped is self._sem_poison
    self.nc.clear_and_free_semaphores(list(self.sems.allocated().values()))
    self.nc.all_engine_barrier()


def _dr2(ap_block):
    # [P, M] -> [P, 2, M] with a stride-0 slot dim: both DoubleRow k-tile
    # slots read the same data (used with a split rhs so the pair sums).
    p, m = ap_block.shape
    return ap_block.rearrange("p (o m) -> p o m", o=1).to_broadcast((p, 2, m))


def _build_kernel(ctx, tc, x8T, x2T, wq8T, wq2T, wk8T, wk2T, wv8T, wv2T,
                  woT, out):
    nc = tc.nc

    wpool = ctx.enter_context(tc.tile_pool(name="weights", bufs=1))
    kvpool = ctx.enter_context(tc.tile_pool(name="kv", bufs=1))
    xpool = ctx.enter_context(tc.tile_pool(name="x", bufs=2))
    qpool = ctx.enter_context(tc.tile_pool(name="q", bufs=2))
    epool = ctx.enter_context(tc.tile_pool(name="exp", bufs=int(os.environ.get("K2_EB", "4"))))
    apool = ctx.enter_context(tc.tile_pool(name="attn", bufs=int(os.environ.get("K2_AB", "2"))))
    opool = ctx.enter_context(tc.tile_pool(name="osb", bufs=int(os.environ.get("K2_OB", "4"))))
    rpool = ctx.enter_context(tc.tile_pool(name="recip", bufs=2))
    # One PSUM pool, 8 banks: sc 2x[128,1024] (4) + at 3x[65|64,512] (3) +
    # fill 1x[128,512] (1).  Projection/O-proj groups share sc/fill slots.
    pp = ctx.enter_context(tc.tile_pool(name="pp", bufs=2, space="PSUM"))

    # --- whole-kernel-resident tiles ---
    wq8 = wpool.tile([128, KC, HD], FP8, tag="wq8")
    wq2 = wpool.tile([128, KC, HD], FP8, tag="wq2")
    wk8 = wpool.tile([128, KC, HD], FP8, tag="wk8")
    wk2 = wpool.tile([128, KC, HD], FP8, tag="wk2")
    wv8 = wpool.tile([128, KC, HD], FP8, tag="wv8")
    wv2 = wpool.tile([128, KC, HD], FP8, tag="wv2")
    wo = wpool.tile([128, 4, D], BF16, tag="wo")
    ones = wpool.tile([1, DK], F32R, tag="ones")
    kT = kvpool.tile([128, 4, S], FP8, tag="kT")
    v = kvpool.tile([128, NKB, HPC, DK + 1], BF16, tag="v")

    def dma_x(j):
        cs = slice(j * CH, (j + 1) * CH)
        x8 = xpool.tile([128, KC, CH], FP8, tag="x8", name=f"x8_{j}")
        x2 = xpool.tile([128, KC, CH], FP8, tag="x2", name=f"x2_{j}")
        nc.sync.dma_start(
            out=x8, in_=x8T[:, cs].rearrange("(c p) n -> p c n", p=128))
        nc.gpsimd.dma_start(
            out=x2, in_=x2T[:, cs].rearrange("(c p) n -> p c n", p=128))
        return x8, x2

    # Priority order on the HWDGE queue: the tensors the first projection
    # groups need come first, one large DMA per tensor.  V-projection
    # inputs ride the SWDGE (gpsimd) path in parallel.
    cs0 = slice(0, CH)
    x8c0 = xpool.tile([128, KC, CH], FP8, tag="x8", name="x8_0")
    x2c0 = xpool.tile([128, KC, CH], FP8, tag="x2", name="x2_0")
    rr = lambda t: t.rearrange("(c p) n -> p c n", p=128)
    # mb0-first: head 0's operands (weight cols 0:128, x chunk 0) land in
    # ~2.5us so the first score pair reaches ACT early; the rest stream in
    # behind on both DGE paths.
    for w, wt in ((wq8, wq8T), (wq2, wq2T), (wk8, wk8T), (wk2, wk2T)):
        nc.sync.dma_start(out=w[:, :, 0:128], in_=rr(wt[:, 0:128]))
    nc.sync.dma_start(out=x8c0, in_=rr(x8T[:, cs0]))
    for w, wt in ((wq8, wq8T), (wq2, wq2T), (wk8, wk8T), (wk2, wk2T)):
        nc.sync.dma_start(out=w[:, :, 128:HD], in_=rr(wt[:, 128:HD]))
    nc.gpsimd.dma_start(out=x2c0, in_=rr(x2T[:, cs0]))
    nc.gpsimd.dma_start(out=wv8, in_=rr(wv8T))
    nc.gpsimd.dma_start(out=wv2, in_=rr(wv2T))
    nc.gpsimd.dma_start(out=wo, in_=rr(woT))
    ones_f32 = wpool.tile([1, DK], F32, tag="ones_f32")
    nc.vector.memset(ones_f32, 1.0)
    nc.vector.tensor_copy(ones, ones_f32)
    vcol_f32 = wpool.tile([128, NKB, HPC, 1], F32, tag="vcol_f32")
    nc.vector.memset(vcol_f32, AL)
    nc.vector.tensor_copy(v[:, :, :, DK:DK + 1], vcol_f32)
    warm = wpool.tile([128, 128], F32R, tag="warm")
    warm_f32 = wpool.tile([128, 128], F32, tag="warm_f32")
    nc.vector.memset(warm_f32, 0.0)
    nc.vector.tensor_copy(warm, warm_f32)
    # preload the ACT exp table set under the input DMAs (~2.7us on HW)
    rcw = rpool.tile([1, DK], F32, tag="rcw", name="rcw", bufs=1)
    nc.scalar.activation(out=rcw, in_=ones_f32, func=EXP, scale=1.0)
    # hold the PE clock-gate open / absorb the cold ramp while DMAs land
    wps = pp.tile([128, 2 * CH], F32, tag="sc", bufs=3, name="wps")
    for r in range(12):
        nc.tensor.matmul(wps[:, (r % 2) * CH:(r % 2) * CH + 128],
                         lhsT=warm, rhs=warm, start=True, stop=True)

    def qkv_fillers(j, x8ch, x2ch):
        cs = slice(j * CH, (j + 1) * CH)
        # split q: slot 0 = fp8(q), slot 1 = fp8(q - slot0)
        qsp = qpool.tile([128, 4, 2, CH], FP8, name=f"qsp{j}", tag="qch")
        fillers = []
        dense = False  # chunk-0 attention overlaps its projections now

        def drgroup(ps, col0, n, seq):
            # one complete PSUM accumulation group over [col0, col0+n):
            # seq = [(lhsT, rhs[:, :, col0:col0+n])...] in contraction order
            for idx, (lh, rh) in enumerate(seq):
                nc.tensor.matmul(
                    ps[:, col0:col0 + n], lhsT=lh,
                    rhs=rh[:, :, col0:col0 + n],
                    start=idx == 0, stop=idx == len(seq) - 1, perf_mode=DR)

        def qkproj2(w8, w2, mb, writer):
            # 2-term fp8 DR: x8*(w8+w2) -> W effectively ~11-bit, q/k carry
            # only the x fp8-quantization noise.  Emitted as two sub-fillers
            # (one per 256-col half) sharing one PSUM fill slot so score
            # pairs can slip in between.
            box = {}

            def half(t):
                if t == 0:
                    box["ps"] = pp.tile([128, CH], F32, tag="sc",
                                        bufs=3, name="psf")
                ps = box["ps"]
                mcols = slice(mb * 128, (mb + 1) * 128)
                seq = [(wt[:, 2 * P:2 * P + 2, mcols],
                        x8ch[:, 2 * P:2 * P + 2, :])
                       for wt in (w8, w2) for P in range(4)]
                drgroup(ps, t * 256, 256, seq)
                if t == 1:
                    writer(ps)
            return [lambda: half(0), lambda: half(1)]

        def qproj(mb):
            def wr(ps):
                nc.vector.tensor_copy(qsp[:, mb, 0, :], ps)
                nc.vector.tensor_sub(qsp[:, mb, 1, :], ps, qsp[:, mb, 0, :])
            return qkproj2(wq8, wq2, mb, wr)

        def kproj(mb):
            return qkproj2(wk8, wk2, mb,
                           lambda ps: nc.vector.tensor_copy(kT[:, mb, cs], ps))

        def vproj(sb):
            box = {}

            def half(t):
                if t == 0:
                    box["ps"] = pp.tile([128, CH], F32, tag="sc",
                                        bufs=3, name="psf")
                ps = box["ps"]
                scols = slice(sb * 128, (sb + 1) * 128)
                seq = [(xt[:, 2 * P:2 * P + 2, scols],
                        wt[:, 2 * P:2 * P + 2, :])
                       for (xt, wt) in [(x8ch, wv8), (x8ch, wv2), (x2ch, wv8)]
                       for P in range(4)]
                drgroup(ps, t * 256, 256, seq)
                if t == 1:
                    sblk = j * 4 + sb
                    nc.vector.tensor_copy(
                        v[:, sblk, :, 0:DK],
                        ps.rearrange("p (h d) -> p h d", h=HPC))
            return [lambda: half(0), lambda: half(1)]

        for mb in range(4):
            fillers.extend(qproj(mb))
        kv = []  # (deadline (h, g) in chunk j's own attention loop, fn)
        for mb in range(4):
            # kT m-block mb is first read by head 2*mb at its step g=2j;
            # deadline at the end of the previous head for copy slack
            for f in kproj(mb):
                kv.append(((max(2 * mb - 1, 0), 98 if mb else 2 * j - 1), f))
        for sb in range(4):
            # v s-block 4j+sb is first read by the pv pair emitted at
            # step g = 2j + sb//2 + 2 of head 0; deadline one step early
            for f in vproj(sb):
                kv.append(((0, 2 * j + sb // 2 + 1), f))
        return qsp, fillers, kv

    def o_fillers(j, ach):
        def oblk(sb, n):
            def f():
                sblk = j * 4 + sb
                osb = opool.tile([128, CH], F32, name="osb", tag="osb")
                ps = pp.tile([128, CH], F32, tag="sc", bufs=3, name="psf")
                for hp in range(4):
                    nc.tensor.matmul(
                        ps, lhsT=ach[:, hp, sb * 128:(sb + 1) * 128],
                        rhs=wo[:, hp, n * CH:(n + 1) * CH],
                        start=(hp == 0), stop=(hp == 3))
                nc.vector.tensor_copy(osb, ps)
                nc.sync.dma_start(
                    out=out[sblk * 128:(sblk + 1) * 128,
                            n * CH:(n + 1) * CH], in_=osb)
            return f
        return [oblk(sb, n) for sb in range(4) for n in range(2)]

    pending_norm = []
    norm_stage2 = []

    def _norm_recip(at_ps, dst):
        # Stage 1 of head normalization: reciprocal of the denominator row.
        # Emitted as soon as the head's PV group closes; the PE broadcast
        # (stage 2) follows a step later so it never waits on the DVE.
        rc = rpool.tile([1, CH], F32R, name="rc", tag="rc", bufs=2)
        with nc.allow_low_precision(reason="f32r feed for PE broadcast"):
            nc.vector.reciprocal(out=rc, in_=at_ps[DK:DK + 1, :])
        norm_stage2.append((at_ps, dst, rc))

    def _norm_bc(at_ps, dst, rc):
        bc = pp.tile([DK, CH], F32, tag="sc", bufs=3, name="bc")
        nc.tensor.matmul(bc, lhsT=ones, rhs=rc, start=True, stop=True)
        # walrus: TensorTensor may read at most one PSUM operand; stage the
        # broadcast through SBUF.
        bcs = rpool.tile([DK, CH], F32, tag="bcs", bufs=1, name="bcs")
        eng = nc.gpsimd if _POOLNORM else nc.vector
        eng.tensor_copy(bcs, bc)
        eng.tensor_mul(dst, at_ps[0:DK, :], bcs)

    from collections import deque
    fillers = deque()   # (None, fn) or ("next", (h, g), fn)
    carry_kv = deque()  # K/V fillers deferred into the current chunk
    carry_next = deque()
    qsp, f0, kv0 = qkv_fillers(0, x8c0, x2c0)
    # Only head 0's operands (q mb0, k mb0) run before the attention loop;
    # everything else is deadline-scheduled into chunk 0's own loop.
    for f in f0[0:2]:
        f()
    for _, f in kv0[0:2]:
        f()
    start_kv = []
    for mb in range(1, 4):
        start_kv.append(((2 * mb - 1, 98), f0[2 * mb]))
        start_kv.append(((2 * mb - 1, 98), f0[2 * mb + 1]))
        start_kv.append(kv0[2 * mb])
        start_kv.append(kv0[2 * mb + 1])
    start_kv.extend(kv0[8:])
    carry_kv.extend(sorted(start_kv, key=lambda ent: ent[0]))

    prev = None  # (j, ach) of the chunk awaiting its O-projection
    for j in range(NCH):
        # stage next chunk's x DMAs + projection fillers, and the previous
        # chunk's O-projection, to fill PE gaps in this ACT-bound phase
        if prev is not None:
            okind = ("kv", (99, 99)) if os.environ.get("K2_ODEF", "0") == "1" \
                else ("now", None)
            fillers.extend((okind[0], okind[1], 853, f)
                           for f in o_fillers(*prev))
        if j + 1 < NCH:
            x8n, x2n = dma_x(j + 1)
            qsp_n, fs, kv_n = qkv_fillers(j + 1, x8n, x2n)
            # q projections of chunk j+1: first needed at (head 2mb, g=-1)
            # of chunk j+1's loop; deadline one head early so the two DVE
            # split-copies complete before the first score matmul reads qsp
            for idx, f in enumerate(fs):
                mb = idx // 2
                dl = (0, -1) if mb == 0 else (2 * mb - 1, 98)
                fillers.append(("kv", dl, 427, f))
            fillers.extend(("kv", dl, 600, f) for dl, f in kv_n)
        else:
            qsp_n = None

        ach = apool.tile([128, 4, CH], BF16, name=f"ach{j}", tag="ach")
        nkb = 4 * (j + 1)
        debt = 0.0
        # cap debt-pops per chunk to its own slack estimate so surplus
        # filler work carries forward to the (filler-starved) last chunk
        pop_cap = _CAP * HPC * (nkb // 2)
        popped_ns = 0.0

        closed = set()

        def _qlo_pv(i):
            # per-block 128-granular trim for PV (bf16: no narrow penalty)
            if i < 4 * j:
                return 0
            return min(128 * (i - 4 * j), CH - 128)

        def _qlo_sc(i):
            # score/exp trim at pair granularity (exp reads written PSUM)
            if i < 4 * j:
                return 0
            return min(128 * (i - 4 * j), CH - 256)

        def emit_pv(ent):
            at_ps, h, pg, pe, is_last = ent
            for t in range(2):
                i = 2 * pg + t
                ql = _qlo_pv(i)
                nc.tensor.matmul(
                    at_ps[:, ql:], lhsT=v[:, i, h, :],
                    rhs=pe[:, t * CH + ql:(t + 1) * CH],
                    start=(i == 0), stop=(is_last and t == 1),
                    skip_group_check=True)
            if is_last:
                closed.add(at_ps.tensor.name)

        def flush_ready():
            # emit norms only for heads whose accumulation group is closed
            # (emission order defines read/write semantics under Tile);
            # stage 2 (PE broadcast) runs one flush behind the reciprocal
            while norm_stage2:
                _norm_bc(*norm_stage2.pop(0))
            while pending_norm and pending_norm[0][0].tensor.name in closed:
                at_ps, dst = pending_norm.pop(0)
                _norm_recip(at_ps, dst)

        pend = []
        for h in range(HPC):
            mb, half = h // 2, h % 2
            row = slice(half * DK, (half + 1) * DK)
            at_ps = pp.tile([DK + 1, CH], F32, tag="at", bufs=2, name="at_ps")
            for g in range(nkb // 2):
                while carry_kv and carry_kv[0][0] <= (h, g):
                    carry_kv.popleft()[1]()
                i0 = 2 * g
                pair_ql = _qlo_sc(i0)  # uniform over the pair so the single
                # exp below reads only written PSUM
                sc = pp.tile([128, 2 * CH], F32, tag="sc", bufs=3, name="sc")
                for t in range(2):
                    i = i0 + t
                    lhs = _dr2(kT[row, mb, i * 128:(i + 1) * 128])
                    w = CH - pair_ql
                    c0 = pair_ql
                    while w > 0:
                        n = min(256, w)
                        nc.tensor.matmul(
                            sc[:, t * CH + c0:t * CH + c0 + n],
                            lhsT=lhs, rhs=qsp[row, mb, :, c0:c0 + n],
                            start=True, stop=True, perf_mode=DR)
                        c0 += n
                        w -= n
                e = epool.tile([128, 2 * CH], BF16, name="e", tag="e")
                sc_v = sc.rearrange("p (t c) -> p t c", t=2)[:, :, pair_ql:]
                e_v = e.rearrange("p (t c) -> p t c", t=2)[:, :, pair_ql:]
                nc.scalar.activation(out=e_v, in_=sc_v, func=EXP,
                                     scale=0.125 / (AL * AL))
                for t in range(2):
                    i = i0 + t
                    if i >= 4 * j:
                        # columns >= 128*(d+1) of the chunk are fully valid
                        # (q > every k in this block); columns < ql are
                        # never read by the trimmed pv.  Mask only between.
                        ql = _qlo_pv(i)
                        hi = min(128 * (i - 4 * j + 1), CH)
                        nc.gpsimd.affine_select(
                            out=e[:, t * CH + ql:t * CH + hi],
                            in_=e[:, t * CH + ql:t * CH + hi],
                            compare_op=mybir.AluOpType.is_ge,
                            fill=0.0, base=j * CH - i * 128 + ql,
                            channel_multiplier=-1, pattern=[[1, hi - ql]])
                # adaptive pacing: fill the PE slack left by the slower exp
                w_sc = 2 * (CH - pair_ql)
                est_exp = 0.8333 * w_sc + 285
                est_inline = 0.5 * 0.4167 * w_sc + 0.4167 * w_sc * 0.93 + 80
                debt += est_exp - est_inline
                while debt > 200 and (carry_kv or (fillers and
                                                   popped_ns < pop_cap)):
                    if carry_kv:
                        ent = carry_kv.popleft()
                        cost = 500
                        ent[-1]()
                    else:
                        ent = fillers.popleft()
                        cost = ent[2]
                        ent[-1]()
                    debt -= cost
                    popped_ns += cost
                if len(pend) > _PVLAG:
                    emit_pv(pend.pop(0))
                flush_ready()
                pend.append((at_ps, h, g, e, g == nkb // 2 - 1))
            pending_norm.append((at_ps, ach[row, mb, :]))
        while pend:
            emit_pv(pend.pop(0))
        flush_ready()
        while norm_stage2:
            _norm_bc(*norm_stage2.pop(0))
        assert not pending_norm
        while carry_kv:
            carry_kv.popleft()[1]()
        # leftover next-chunk K/V/Q fillers defer into chunk j+1's loop
        # (deadline-scheduled) instead of serializing at the boundary;
        # leftover O-projections must finish now (ach slot rotation).
        for ent in fillers:
            if ent[0] == "kv":
                carry_next.append((ent[1], ent[-1]))
            else:
                ent[-1]()
        fillers.clear()
        carry_kv = deque(sorted(carry_next, key=lambda e: e[0]))
        carry_next = deque()
        prev = (j, ach)
        qsp = qsp_n

    jf, achf = prev
    for sb in range(4):
        sblk = jf * 4 + sb
        for n in range(2):
            ps = pp.tile([128, CH], F32, tag="sc", bufs=3, name="ps_of")
            for hp in range(4):
                nc.tensor.matmul(
                    ps, lhsT=achf[:, hp, sb * 128:(sb + 1) * 128],
                    rhs=wo[:, hp, n * CH:(n + 1) * CH],
                    start=(hp == 0), stop=(hp == 3))
            osb = opool.tile([128, CH], F32, name="osb", tag="osb")
            nc.vector.tensor_copy(osb, ps)
            nc.sync.dma_start(
                out=out[sblk * 128:(sblk + 1) * 128, n * CH:(n + 1) * CH],
                in_=osb)


def _split_excess_waits(nc, max_waits=1):
    # This walrus build rejects instructions carrying more than a couple of
    # sem waits ("Too many sync wait commands").  Engines execute their
    # stream in order, so excess waits can be moved onto nofuse nops placed
    # immediately before the instruction on the same engine.
    ctr = 0
    for blk in nc.m.functions[0].blocks:
        insts = blk.instructions
        out = []
        changed = False
        for inst in insts:
            si = inst.sync_info
            if si is not None and si.on_wait and len(si.on_wait) > max_waits:
                waits = list(si.on_wait)
                extra, keep = waits[:-max_waits], waits[-max_waits:]
                for gi in range(0, len(extra), max_waits):
                    ctr += 1
                    out.append(mybir.InstNoOp(
                        name=f"wsplit_{ctr}",
                        engine=inst.engine,
                        bass_nofuse=True,
                        sync_info=mybir.SyncInfo(
                            on_wait=extra[gi:gi + max_waits], on_update=[]),
                    ))
                inst.sync_info = mybir.SyncInfo(
                    on_wait=keep, on_update=si.on_update)
                changed = True
            out.append(inst)
        if changed:
            insts[:] = out


_CACHE = {}


def _get_nc(split=True):
    if "nc" in _CACHE:
        return _CACHE["nc"]
    tile.TileContext._drain_and_barrier = _drain_and_barrier_split
    nc = bass.Bass("TRN2", target_bir_lowering=False, debug=False)
    x8T = nc.dram_tensor("x8T", [D, S], FP8, kind="ExternalInput").ap()
    x2T = nc.dram_tensor("x2T", [D, S], FP8, kind="ExternalInput").ap()
    wq8T = nc.dram_tensor("wq8T", [D, HD], FP8, kind="ExternalInput").ap()
    wq2T = nc.dram_tensor("wq2T", [D, HD], FP8, kind="ExternalInput").ap()
    wk8T = nc.dram_tensor("wk8T", [D, HD], FP8, kind="ExternalInput").ap()
    wk2T = nc.dram_tensor("wk2T", [D, HD], FP8, kind="ExternalInput").ap()
    wv8T = nc.dram_tensor("wv8T", [D, HD], FP8, kind="ExternalInput").ap()
    wv2T = nc.dram_tensor("wv2T", [D, HD], FP8, kind="ExternalInput").ap()
    woT = nc.dram_tensor("woT", [HD, D], BF16, kind="ExternalInput").ap()
    out = nc.dram_tensor("out", [S, D], F32, kind="ExternalOutput").ap()
    from contextlib import ExitStack
    with tile.TileContext(nc) as tc, ExitStack() as ctx:
        _build_kernel(ctx, tc, x8T, x2T, wq8T, wq2T, wk8T, wk2T, wv8T, wv2T,
                      woT, out)
    if split:
        _split_excess_waits(nc)
        _CACHE["nc"] = nc
    return nc


def _fp8_split(a):
    hi = np.asarray(a, NP_FP8)
    lo = np.asarray(a - hi.astype(np.float32), NP_FP8)
    return hi, lo


def make_in_maps(x, Wq, Wk, Wv, Wo):
    x = np.asarray(x, np.float32)
    Wq, Wk, Wv, Wo = (np.asarray(w, np.float32) for w in (Wq, Wk, Wv, Wo))
    in_maps = []
    for c in range(8):
        b, hh = c // 2, c % 2
        cols = slice(hh * HD, (hh + 1) * HD)
        xT = np.ascontiguousarray(x[b].T)
        x8, x2 = _fp8_split(xT)
        wq8, wq2 = _fp8_split(np.ascontiguousarray(Wq[cols, :].T) * AL)
        wk8, wk2 = _fp8_split(np.ascontiguousarray(Wk[cols, :].T) * AL)
        wv8, wv2 = _fp8_split(np.ascontiguousarray(Wv[cols, :].T) * AL)
        in_maps.append({
            "x8T": x8,
            "x2T": x2,
            "wq8T": wq8,
            "wq2T": wq2,
            "wk8T": wk8,
            "wk2T": wk2,
            "wv8T": wv8,
            "wv2T": wv2,
            "woT": np.ascontiguousarray(Wo[:, cols].T).astype(NP_BF16),
        })
    return in_maps


def kernel(x, Wq, Wk, Wv, Wo, _trace=False, _trace_kwargs=None):
    nc = _get_nc()
    in_maps = make_in_maps(x, Wq, Wk, Wv, Wo)
    res = run_bass_kernel_spmd(
        nc, in_maps, core_ids=list(range(8)), trace=_trace,
        **(_trace_kwargs or {}))
    outs = [res.results[c]["out"] for c in range(8)]
    full = np.stack([outs[2 * b] + outs[2 * b + 1] for b in range(B)])
    if _trace:
        _CACHE["last_results"] = res
    return full.astype(np.float32)
